# revision 25
# baseline (speedup 1.0000x reference)
"""Trainium2 Bass kernel: rFFT(65536)->keep 4000 bins->LayerNorm(8000)->Linear(8000,512)->SiLU.

Math: 2-level pruned Cooley-Tukey. n = 512*n1 + n2 (n1 in [0,128), n2 in [0,512)).
  k1 = k mod 128, q = k div 128; keep k < 4000 -> q in [0,32), mask (q=31, k1>=32).
  Pair rows (xa, xb) as complex z = xa + i*xb:
  Y[n2,k1]  = sum_n1 z[512*n1+n2] * exp(-2i pi n1 k1 / 128)        (inner DFT, f32r matmul)
  Twiddle T[n2,k1] = exp(-2i pi n2 k1 / 65536) and outer DFT over n2 fused via
  3-mult Karatsuba: m1 = Yre*Tc, m2 = Yim*Ts, m3 = (Yre+Yim)*(Tc+Ts) (DVE/Pool), then
  X[q,k1] = sum_n2 m1*(Wc+Ws | Ws-Wc) + m2*(Ws-Wc | -(Ws+Wc)) + m3*(-Ws | Wc)
  for 64 q (0..31 and 480..511), 12 matmul accumulation steps (was 16).
  Separation uses 2 shared mirror terms per q (a_re/b_im share M_re, a_im/b_re
  share M_im), 4 matmuls per q grouped by stationary, then 4 fused STTs per
  4-q group. LayerNorm folded into the linear:
  out = SiLU( (G - mu*c) * istd + d ),  G = s @ A',  A'[e,o] = ln_w[e]*W[o,e],
  c = sum_e ln_w*W, d = ln_b @ W.T + b. sum(s) rides the linear as an extra
  N=1 matmul on the same stationary; sum(s^2) via DVE square+reduce.
"""

import numpy as np
import ml_dtypes

import concourse.bass as bass
import concourse.tile as tile
from concourse import bacc, mybir
from concourse.bass_utils import run_bass_kernel_spmd

N_CORES = 8
B_FULL = 2048
FFT_N = 65536
N1 = 128      # inner DFT length; k1 = k mod 128
N2 = 512      # outer length; q = k div 128
KEEP = 4000
QK = 32       # q in [0, 32), 32*128 = 4096 bins computed, 96 masked
EPS = 1e-5

f32 = mybir.dt.float32
f32r = mybir.dt.float32r
bf16 = mybir.dt.bfloat16
ALU = mybir.AluOpType
ACT = mybir.ActivationFunctionType
BF16 = ml_dtypes.bfloat16


# ---------------------------------------------------------------- host consts
def _host_consts():
    n1 = np.arange(N1, dtype=np.float64)
    k1 = np.arange(N1, dtype=np.float64)
    n2 = np.arange(N2, dtype=np.float64)

    ang1 = 2.0 * np.pi * np.outer(n1, k1) / N1
    C1, S1 = np.cos(ang1), -np.sin(ang1)
    # pair-packed inner: y = [Yre | Yim] accumulates
    #   lhsT=xa with [C1 | S1]  plus  lhsT=xb with [-S1 | C1]
    f1 = np.concatenate([np.concatenate([C1, S1], axis=1),
                         np.concatenate([-S1, C1], axis=1)], axis=0).reshape(
        2, 128, 256)  # f1[0]=for xa, f1[1]=for xb

    angT = 2.0 * np.pi * np.outer(n2, k1) / FFT_N               # [512,128]
    Tc, Ts = np.cos(angT), -np.sin(angT)
    # chunk-major twiddle tables [n2', c*128 + k1]: Tc | Ts | Tc+Ts
    tcs = np.zeros((128, 3 * 512))
    for c in range(4):
        sl = slice(c * 128, (c + 1) * 128)
        tcs[:, 0 + c * 128:0 + (c + 1) * 128] = Tc[sl]
        tcs[:, 512 + c * 128:512 + (c + 1) * 128] = Ts[sl]
        tcs[:, 1024 + c * 128:1024 + (c + 1) * 128] = Tc[sl] + Ts[sl]

    qall = np.concatenate([np.arange(QK), np.arange(N2 - QK, N2)])  # 64 q values
    angW = 2.0 * np.pi * np.outer(n2, qall.astype(np.float64)) / N2  # [512,64]
    WcT, WsT = np.cos(angW), -np.sin(angW)
    # Karatsuba outer weights, 3 blocks of 128 cols (64 re | 64 im) per chunk
    wqk = np.zeros((128, 4 * 384))
    for c in range(4):
        Wc = WcT[c * 128:(c + 1) * 128]; Ws = WsT[c * 128:(c + 1) * 128]
        base = c * 384
        wqk[:, base + 0:base + 64] = Wc + Ws
        wqk[:, base + 64:base + 128] = Ws - Wc
        wqk[:, base + 128:base + 192] = Ws - Wc
        wqk[:, base + 192:base + 256] = -(Ws + Wc)
        wqk[:, base + 256:base + 320] = -Ws
        wqk[:, base + 320:base + 384] = Wc

    # separation mirror constants, scaled 0.5, as matmul lhsT [src_k1, dst_k1]
    R = np.zeros((128, 128)); S0 = np.zeros((128, 128))
    for d in range(1, 128):
        R[128 - d, d] = 1.0
    S0[0, 0] = 1.0
    rev2 = np.concatenate([0.5 * R, 0.5 * S0], axis=1)  # [128, 256]

    return (f1.astype(np.float32), tcs.astype(BF16), wqk.astype(BF16),
            rev2.astype(BF16))


def _host_linear(ln_w, ln_b, W, b):
    # A'[e,o] = ln_w[e] * W[o,e]; permuted to my (j,k1) layout with masked tail.
    Af = (ln_w[None, :] * W).T.astype(np.float64)               # [8000, 512]
    Ap = np.zeros((8192, 512))
    for j in range(64):
        for_k1 = np.arange(128)
        if j < 32:
            e = j * 128 + for_k1
            valid = e < KEEP
        else:
            e = KEEP + (j - 32) * 128 + for_k1
            valid = e < 2 * KEEP
        Ap[j * 128 + for_k1[valid]] = Af[e[valid]]
    # SBUF layout ap_w[k1, j*512 + o]
    apw = Ap.reshape(64, 128, 512).transpose(1, 0, 2).reshape(128, 64 * 512)
    cvec = (ln_w[None, :] * W).sum(axis=1)                      # [512]
    dvec = ln_b @ W.T + b                                       # [512]
    cb = np.tile(cvec.astype(np.float32)[None, :], (128, 1))
    db = np.tile(dvec.astype(np.float32)[None, :], (128, 1))
    return apw.astype(BF16), cb, db


# ---------------------------------------------------------------- bass kernel
def build_nc(rows, block, reps=1):
    """Build the per-core Bass program for `rows` batch rows, processed in
    groups of `block` (the LN/linear batch tile, <= 128). reps>1 repeats the
    whole computation back-to-back (for HW timing)."""
    assert rows % block == 0
    nblk = rows // block
    nc = bacc.Bacc("TRN2", target_bir_lowering=False, debug=False)

    xd = nc.dram_tensor("x", [rows // 2, 128, 1024], f32r, kind="ExternalInput")
    f1d = nc.dram_tensor("f1", [2, 128, 256], f32r, kind="ExternalInput")
    tcsd = nc.dram_tensor("tcs", [128, 1536], bf16, kind="ExternalInput")
    wqd = nc.dram_tensor("wqk", [128, 1536], bf16, kind="ExternalInput")
    revd = nc.dram_tensor("rev2", [128, 256], bf16, kind="ExternalInput")
    apwd = nc.dram_tensor("apw", [128, 64 * 512], bf16, kind="ExternalInput")
    cd = nc.dram_tensor("cvec", [128, 512], f32, kind="ExternalInput")
    dd = nc.dram_tensor("dvec", [128, 512], f32, kind="ExternalInput")
    outd = nc.dram_tensor("out", [nblk, block, 512], f32, kind="ExternalOutput")

    from contextlib import ExitStack
    with tile.TileContext(nc) as tc, ExitStack() as es:
        consts = es.enter_context(tc.tile_pool(name="consts", bufs=1))
        f1a_sb = consts.tile([128, 256], f32r, name="f1a_sb")
        f1b_sb = consts.tile([128, 256], f32r, name="f1b_sb")
        tcs_sb = consts.tile([128, 1536], bf16, name="tcs_sb")
        wq_sb = consts.tile([128, 1536], bf16, name="wq_sb")
        rev_sb = consts.tile([128, 256], bf16, name="rev_sb")
        apw_sb = consts.tile([128, 64 * 512], bf16, name="apw_sb")
        c_sb = consts.tile([128, 512], f32, name="c_sb")
        d_sb = consts.tile([128, 512], f32, name="d_sb")
        ones_sb = consts.tile([128, 1], f32, name="ones_sb")
        ones_bf = consts.tile([128, 1], bf16, name="ones_bf")
        # keep the SP queue exclusively for x row loads: f1 (needed by the
        # first matmul, tiny) leads, medium consts ride the Activation HWDGE
        # queue, and the big linear weights (needed ~100us later) go via
        # gpsimd SWDGE
        nc.sync.dma_start(out=f1a_sb, in_=f1d[0])
        nc.sync.dma_start(out=f1b_sb, in_=f1d[1])
        for sb, dr in ((tcs_sb, tcsd), (wq_sb, wqd), (rev_sb, revd)):
            nc.scalar.dma_start(out=sb, in_=dr[:])
        for sb, dr in ((apw_sb, apwd), (c_sb, cd), (d_sb, dd)):
            nc.gpsimd.dma_start(out=sb, in_=dr[:])
        nc.vector.memset(ones_sb, 1.0)
        nc.vector.memset(ones_bf, 1.0)

        xp = es.enter_context(tc.tile_pool(name="xp", bufs=6))
        yp = es.enter_context(tc.tile_pool(name="yp", bufs=2, space="PSUM"))
        ybp = es.enter_context(tc.tile_pool(name="ybp", bufs=3))
        mp = es.enter_context(tc.tile_pool(name="mp", bufs=2))
        op1 = es.enter_context(tc.tile_pool(name="op1", bufs=1, space="PSUM"))
        pm = es.enter_context(tc.tile_pool(name="pm", bufs=1, space="PSUM"))
        svp = es.enter_context(tc.tile_pool(name="svp", bufs=2))
        sp = es.enter_context(tc.tile_pool(name="sp", bufs=2))
        sqp = es.enter_context(tc.tile_pool(name="sqp", bufs=1))
        stp = es.enter_context(tc.tile_pool(name="stp", bufs=2))
        gp = es.enter_context(tc.tile_pool(name="gp", bufs=1, space="PSUM"))
        g2p = es.enter_context(tc.tile_pool(name="g2p", bufs=1, space="PSUM"))
        smp = es.enter_context(tc.tile_pool(name="smp", bufs=2))
        ep = es.enter_context(tc.tile_pool(name="ep", bufs=2))

        apw3 = apw_sb.rearrange("p (j o) -> p j o", j=64)
        tc3 = tcs_sb.rearrange("p (v c k) -> p v c k", v=3, c=4)

        pbk = block // 2  # pairs per block

        def _tail_pieces(blk, s_buf, g_ps, g2):
            """Stats + LN + SiLU + output DMA for a finished block, split into
            small pieces so they interleave with the next block's pairs.
            sum(s^2) = DVE squares (bf16 2x) + an N=1 matmul series on PE
            accumulating over k1 and j at once (pm bank is free then)."""
            sq_hold = {}
            stat_hold = {}

            def sq_half(half):
                sq = sqp.tile([128, 32 * block], bf16, name="sq_buf")
                sq_hold[half] = sq
                sh = s_buf[:, half * 32 * block:(half + 1) * 32 * block]
                nc.vector.tensor_mul(sq, sh, sh)

            def sq_stats(half):
                if half == 0:
                    stat_hold[0] = pm.tile([128, 4], f32, name="stat_ps",
                                           tag="psm")
                stat_ps = stat_hold[0]
                sq = sq_hold[half]
                for jj in range(32):
                    nc.tensor.matmul(
                        stat_ps[:block, 0:1],
                        lhsT=sq[:, jj * block:(jj + 1) * block], rhs=ones_bf,
                        start=(half == 0 and jj == 0),
                        stop=(half == 1 and jj == 31))

            def finale():
                stat_ps = stat_hold[0]
                mu = smp.tile([128, 1], f32, name="mu")
                negmu = smp.tile([128, 1], f32, name="negmu")
                e2 = smp.tile([128, 1], f32, name="e2")
                varep = smp.tile([128, 1], f32, name="varep")
                rec = smp.tile([128, 1], f32, name="rec")
                istd = smp.tile([128, 1], f32, name="istd")
                nc.vector.tensor_scalar_mul(mu[:block], g2[:block],
                                            1.0 / (2 * KEEP))
                nc.vector.tensor_scalar_mul(negmu[:block], g2[:block],
                                            -1.0 / (2 * KEEP))
                nc.vector.tensor_scalar_mul(e2[:block], stat_ps[:block, 0:1],
                                            1.0 / (2 * KEEP))
                # varep = e2 - mu^2 + EPS = (mu * -mu) + e2, then + EPS
                nc.vector.scalar_tensor_tensor(
                    out=varep[:block], in0=mu[:block], scalar=negmu[:block],
                    in1=e2[:block], op0=ALU.mult, op1=ALU.add)
                nc.vector.tensor_scalar_add(varep[:block], varep[:block], EPS)
                nc.vector.reciprocal(rec[:block], varep[:block])
                nc.scalar.activation(istd[:block], rec[:block], ACT.Sqrt)
                p1 = ep.tile([128, 512], f32, name="p1")
                p2 = ep.tile([128, 512], f32, name="p2")
                o_sb = ep.tile([128, 512], f32, name="o_sb")
                nc.vector.scalar_tensor_tensor(
                    out=p1[:block], in0=c_sb[:block], scalar=negmu[:block],
                    in1=g_ps[:block], op0=ALU.mult, op1=ALU.add)
                nc.vector.scalar_tensor_tensor(
                    out=p2[:block], in0=p1[:block], scalar=istd[:block],
                    in1=d_sb[:block], op0=ALU.mult, op1=ALU.add)
                nc.scalar.activation(o_sb[:block], p2[:block], ACT.Silu)
                nc.scalar.dma_start(out=outd[blk], in_=o_sb[:block])

            return [lambda: sq_half(0), lambda: sq_stats(0),
                    lambda: sq_half(1), lambda: sq_stats(1), finale]

        import contextlib
        loop_ctx = tc.For_i(0, reps, 1) if reps > 1 else contextlib.nullcontext()
        with loop_ctx:
          pending = iter(())
          for blk in range(nblk):
              sv_buf = svp.tile([128, 128 * pbk], bf16, name="sv_buf")
              sv4 = sv_buf.rearrange("p (jq t) -> p jq t", jq=128)
              s_buf = sp.tile([128, 64 * block], bf16, name="s_buf")
              s4 = s_buf.rearrange("p (j t u) -> p j t u", j=64, u=2)
              for p in range(pbk):
                  if p >= 6 and (p - 6) % 8 == 0:
                      piece = next(pending, None)
                      if piece is not None:
                          piece()
                  xab = xp.tile([128, 1024], f32r, name="xab")
                  nc.sync.dma_start(out=xab, in_=xd[blk * pbk + p])
                  y_ps = yp.tile([128, 1024], f32, name="y_ps")
                  for c in range(4):
                      reg = y_ps[:, c * 256:(c + 1) * 256]
                      nc.tensor.matmul(reg, lhsT=xab[:, c * 128:(c + 1) * 128],
                                       rhs=f1a_sb, start=True, stop=False)
                      nc.tensor.matmul(reg,
                                       lhsT=xab[:, 512 + c * 128:512 + (c + 1) * 128],
                                       rhs=f1b_sb, start=False, stop=True)
                  y_bf = ybp.tile([128, 1024], bf16, name="y_bf")
                  nc.scalar.copy(out=y_bf, in_=y_ps)
                  y4 = y_bf.rearrange("p (c t k) -> p c t k", c=4, t=2)
                  yre = y4[:, :, 0, :]
                  yim = y4[:, :, 1, :]
                  m1 = mp.tile([128, 512], bf16, name="m1")
                  m2 = mp.tile([128, 512], bf16, name="m2")
                  m3 = mp.tile([128, 512], bf16, name="m3")
                  msum = mp.tile([128, 512], bf16, name="msum")
                  m1_3 = m1.rearrange("p (c k) -> p c k", c=4)
                  m2_3 = m2.rearrange("p (c k) -> p c k", c=4)
                  m3_3 = m3.rearrange("p (c k) -> p c k", c=4)
                  ms_3 = msum.rearrange("p (c k) -> p c k", c=4)
                  nc.vector.tensor_mul(m1_3, yre, tc3[:, 0])
                  nc.vector.tensor_mul(m2_3, yim, tc3[:, 1])
                  nc.vector.tensor_add(ms_3, yre, yim)
                  nc.gpsimd.tensor_mul(m3_3, ms_3, tc3[:, 2])
                  # outer DFT (64 q: 0..31 and 480..511), 12-matmul accumulation
                  # o cols = [re-lo 0:32 | re-hi 32:64 | im-lo 64:96 | im-hi 96:128]
                  osl = op1.tile([128, 128], f32, name="o_t")
                  # m1/m2 (DVE-computed) first; slower Pool m3 accumulates last
                  for mi, mt in ((0, m1), (1, m2)):
                      for c in range(4):
                          nc.tensor.matmul(
                              osl, lhsT=mt[:, c * 128:(c + 1) * 128],
                              rhs=wq_sb[:, c * 384 + mi * 128:c * 384 + mi * 128 + 128],
                              start=(mi == 0 and c == 0), stop=False)
                  for c in range(4):
                      nc.tensor.matmul(
                          osl, lhsT=m3[:, c * 128:(c + 1) * 128],
                          rhs=wq_sb[:, c * 384 + 256:c * 384 + 384],
                          start=False, stop=(c == 3))
                  nc.scalar.copy(out=sv4[:, :, p:p + 1], in_=osl.unsqueeze(2))
              # ---- separation: s_a = (X[k] + conj(X[-k]))/2, s_b = (X[k] -
              # conj(X[-k]))/(2i). Two shared mirror terms per q:
              #   M_re = 0.5*R@sv[63-q] + 0.5*S0@sv[64-q or 0]
              #   M_im = 0.5*R@sv[127-q] + 0.5*S0@sv[128-q or 64]
              # a_re = 0.5 sv[q] + M_re       a_im = 0.5 sv[64+q] - M_im
              # b_re = 0.5 sv[64+q] + M_im    b_im = -0.5 sv[q] + M_re
              RP, S0P = rev_sb[:, 0:128], rev_sb[:, 128:256]
              # linear psum (G) + sum(s) riding the same stationary as an N=1
              # matmul; the j-chunks are emitted interleaved with the sep
              # groups that produce them so PE fills the sep STT latency
              g_ps = gp.tile([128, 512], f32, name="g_ps")
              g2 = g2p.tile([128, 1], f32, name="g2")
              for g in range(9):
                if g < 8:
                  q0 = 4 * g
                  psm = pm.tile([128, 8 * pbk], f32, name="psm", tag="psm")
                  # M_re(q) and M_im(q) share lhsT and their sv columns sit 64
                  # jq apart -> one stride-64 rhs AP covers both (N=2*pbk);
                  # one pending accumulation group per psum bank at a time:
                  # R (start) then its S0 (stop) before the next region
                  for t in range(4):
                      q = q0 + t
                      reg = psm[:, t * 2 * pbk:(t + 1) * 2 * pbk]
                      nc.tensor.matmul(reg, lhsT=RP,
                                       rhs=sv4[:, 63 - q::64, :],
                                       start=True, stop=False)
                      nc.tensor.matmul(reg, lhsT=S0P,
                                       rhs=sv4[:, (0 if q == 0 else 64 - q)::64, :],
                                       start=False, stop=True)
                  psm5 = psm.rearrange("p (t m b) -> p t m b", t=4, m=2)
                  psm_re = psm5[:, :, 0, :].unsqueeze(3)
                  psm_im = psm5[:, :, 1, :].unsqueeze(3)
                  nc.vector.scalar_tensor_tensor(
                      out=s4[:, q0:q0 + 4, :, 0:1],
                      in0=sv4[:, q0:q0 + 4, :].unsqueeze(3), scalar=0.5,
                      in1=psm_re, op0=ALU.mult, op1=ALU.add)
                  nc.vector.scalar_tensor_tensor(
                      out=s4[:, 32 + q0:32 + q0 + 4, :, 0:1],
                      in0=sv4[:, 64 + q0:64 + q0 + 4, :].unsqueeze(3), scalar=0.5,
                      in1=psm_im, op0=ALU.mult, op1=ALU.subtract)
                  nc.vector.scalar_tensor_tensor(
                      out=s4[:, q0:q0 + 4, :, 1:2],
                      in0=sv4[:, 64 + q0:64 + q0 + 4, :].unsqueeze(3), scalar=0.5,
                      in1=psm_im, op0=ALU.mult, op1=ALU.add)
                  nc.vector.scalar_tensor_tensor(
                      out=s4[:, 32 + q0:32 + q0 + 4, :, 1:2],
                      in0=sv4[:, q0:q0 + 4, :].unsqueeze(3), scalar=-0.5,
                      in1=psm_re, op0=ALU.mult, op1=ALU.add)
                  if g == 7:
                      # mask bins k >= 4000 (q = 31, k1 >= 32), re and im;
                      # gpsimd keeps these off the busier DVE queue
                      for pbase in (32, 64, 96):
                          nc.gpsimd.memset(
                              s_buf[pbase:pbase + 32, 31 * block:32 * block], 0.0)
                          nc.gpsimd.memset(
                              s_buf[pbase:pbase + 32, 63 * block:64 * block], 0.0)
                if g >= 1:
                  # linear j-chunk the PREVIOUS sep group completed, staggered
                  # so PE never waits on this group's STTs (emission order
                  # makes j=0 the accumulation start and j=63 the stop)
                  q0p = 4 * (g - 1)
                  for j in (list(range(q0p, q0p + 4))
                            + list(range(32 + q0p, 32 + q0p + 4))):
                      sj = s_buf[:, j * block:(j + 1) * block]
                      nc.tensor.matmul(g_ps[:block], lhsT=sj, rhs=apw3[:, j, :],
                                       start=(j == 0), stop=(j == 63))
                      nc.tensor.matmul(g2[:block], lhsT=sj, rhs=ones_bf,
                                       start=(j == 0), stop=(j == 63))
              # defer this block's stats/LN tail: emitted in small pieces
              # between the NEXT block's pairs so the DVE queue never bursts
              pending = iter(_tail_pieces(blk, s_buf, g_ps, g2))
          for piece in pending:
              piece()

    nc.compile()
    return nc


# ---------------------------------------------------------------- entry points
_CACHE = {}


def _get_nc(rows, block, reps=1):
    key = (rows, block, reps)
    if key not in _CACHE:
        _CACHE[key] = build_nc(rows, block, reps)
    return _CACHE[key]


def run_cores(x, ln_w, ln_b, W, b, rows_per_core, block, n_cores=N_CORES,
              trace=False):
    f1, tcs, wqk, rev2 = _host_consts()
    apw, cb, db = _host_linear(
        np.asarray(ln_w, np.float64), np.asarray(ln_b, np.float64),
        np.asarray(W, np.float64), np.asarray(b, np.float64))
    nc = _get_nc(rows_per_core, block)
    x = np.ascontiguousarray(np.asarray(x, np.float32))
    in_maps = []
    for i in range(n_cores):
        xs = x[i * rows_per_core:(i + 1) * rows_per_core].reshape(
            rows_per_core // 2, 2, 128, 512).transpose(0, 2, 1, 3).reshape(
            rows_per_core // 2, 128, 1024)
        in_maps.append({
            "x": np.ascontiguousarray(xs), "f1": f1, "tcs": tcs, "wqk": wqk,
            "rev2": rev2, "apw": apw, "cvec": cb, "dvec": db,
        })
    res = run_bass_kernel_spmd(nc, in_maps, core_ids=list(range(n_cores)),
                               trace=trace)
    outs = [res.results[i]["out"].reshape(rows_per_core, 512)
            for i in range(n_cores)]
    return np.concatenate(outs, axis=0), res


def kernel(x, ln_w, ln_b, W, b):
    rows = B_FULL // N_CORES
    out, _ = run_cores(x, ln_w, ln_b, W, b, rows, 128)
    return out.reshape(B_FULL, 1, 512).astype(np.float32)


# revision 26
# speedup vs baseline: 1.4065x; 1.4065x over previous
"""Trainium2 Bass kernel: rFFT(65536)->keep 4000 bins->LayerNorm(8000)->Linear(8000,512)->SiLU.

Math: 2-level pruned Cooley-Tukey. n = 512*n1 + n2 (n1 in [0,128), n2 in [0,512)).
  k1 = k mod 128, q = k div 128; keep k < 4000 -> q in [0,32), mask (q=31, k1>=32).
  Pair rows (xa, xb) as complex z = xa + i*xb:
  Y[n2,k1]  = sum_n1 z[512*n1+n2] * exp(-2i pi n1 k1 / 128)        (inner DFT, f32r matmul)
  Twiddle T[n2,k1] = exp(-2i pi n2 k1 / 65536) and outer DFT over n2 fused via
  3-mult Karatsuba: m1 = Yre*Tc, m2 = Yim*Ts, m3 = (Yre+Yim)*(Tc+Ts) (DVE/Pool), then
  X[q,k1] = sum_n2 m1*(Wc+Ws | Ws-Wc) + m2*(Ws-Wc | -(Ws+Wc)) + m3*(-Ws | Wc)
  for 64 q (0..31 and 480..511), 12 matmul accumulation steps (was 16).
  Separation uses 2 shared mirror terms per q (a_re/b_im share M_re, a_im/b_re
  share M_im); M_re/M_im ride one stride-64 rhs AP -> 2 matmuls per q, then 4
  fused STTs per 4-q group. The linear j-chunks are emitted staggered into the
  separation groups that produce them, and each block's stats/LN/SiLU tail is
  deferred and drip-fed between the next block's pairs so no engine bursts.
  LayerNorm folded into the linear:
  out = SiLU( (G - mu*c) * istd + d ),  G = s @ A',  A'[e,o] = ln_w[e]*W[o,e],
  c = sum_e ln_w*W, d = ln_b @ W.T + b. sum(s) and sum(s^2) ride N=1 matmul
  series (sum(s) on the linear's stationary; sum(s^2) on DVE-squared tiles).
"""

import numpy as np
import ml_dtypes

import concourse.bass as bass
import concourse.tile as tile
from concourse import bacc, mybir
from concourse.bass_utils import run_bass_kernel_spmd

N_CORES = 8
B_FULL = 2048
FFT_N = 65536
N1 = 128      # inner DFT length; k1 = k mod 128
N2 = 512      # outer length; q = k div 128
KEEP = 4000
QK = 32       # q in [0, 32), 32*128 = 4096 bins computed, 96 masked
EPS = 1e-5

f32 = mybir.dt.float32
f32r = mybir.dt.float32r
bf16 = mybir.dt.bfloat16
ALU = mybir.AluOpType
ACT = mybir.ActivationFunctionType
BF16 = ml_dtypes.bfloat16


# ---------------------------------------------------------------- host consts
def _host_consts():
    n1 = np.arange(N1, dtype=np.float64)
    k1 = np.arange(N1, dtype=np.float64)
    n2 = np.arange(N2, dtype=np.float64)

    ang1 = 2.0 * np.pi * np.outer(n1, k1) / N1
    C1, S1 = np.cos(ang1), -np.sin(ang1)
    # pair-packed inner: y = [Yre | Yim] accumulates
    #   lhsT=xa with [C1 | S1]  plus  lhsT=xb with [-S1 | C1]
    f1 = np.concatenate([np.concatenate([C1, S1], axis=1),
                         np.concatenate([-S1, C1], axis=1)], axis=0).reshape(
        2, 128, 256)  # f1[0]=for xa, f1[1]=for xb

    angT = 2.0 * np.pi * np.outer(n2, k1) / FFT_N               # [512,128]
    Tc, Ts = np.cos(angT), -np.sin(angT)
    # chunk-major twiddle tables [n2', c*128 + k1]: Tc | Ts | Tc+Ts
    tcs = np.zeros((128, 3 * 512))
    for c in range(4):
        sl = slice(c * 128, (c + 1) * 128)
        tcs[:, 0 + c * 128:0 + (c + 1) * 128] = Tc[sl]
        tcs[:, 512 + c * 128:512 + (c + 1) * 128] = Ts[sl]
        tcs[:, 1024 + c * 128:1024 + (c + 1) * 128] = Tc[sl] + Ts[sl]

    qall = np.concatenate([np.arange(QK), np.arange(N2 - QK, N2)])  # 64 q values
    angW = 2.0 * np.pi * np.outer(n2, qall.astype(np.float64)) / N2  # [512,64]
    WcT, WsT = np.cos(angW), -np.sin(angW)
    # Karatsuba outer weights, 3 blocks of 128 cols (64 re | 64 im) per chunk
    wqk = np.zeros((128, 4 * 384))
    for c in range(4):
        Wc = WcT[c * 128:(c + 1) * 128]; Ws = WsT[c * 128:(c + 1) * 128]
        base = c * 384
        wqk[:, base + 0:base + 64] = Wc + Ws
        wqk[:, base + 64:base + 128] = Ws - Wc
        wqk[:, base + 128:base + 192] = Ws - Wc
        wqk[:, base + 192:base + 256] = -(Ws + Wc)
        wqk[:, base + 256:base + 320] = -Ws
        wqk[:, base + 320:base + 384] = Wc

    # separation mirror constants, scaled 0.5, as matmul lhsT [src_k1, dst_k1]
    R = np.zeros((128, 128)); S0 = np.zeros((128, 128))
    for d in range(1, 128):
        R[128 - d, d] = 1.0
    S0[0, 0] = 1.0
    rev2 = np.concatenate([0.5 * R, 0.5 * S0], axis=1)  # [128, 256]

    return (f1.astype(np.float32), tcs.astype(BF16), wqk.astype(BF16),
            rev2.astype(BF16))


def _host_linear(ln_w, ln_b, W, b):
    # A'[e,o] = ln_w[e] * W[o,e]; permuted to my (j,k1) layout with masked tail.
    Af = (ln_w[None, :] * W).T.astype(np.float64)               # [8000, 512]
    Ap = np.zeros((8192, 512))
    for j in range(64):
        for_k1 = np.arange(128)
        if j < 32:
            e = j * 128 + for_k1
            valid = e < KEEP
        else:
            e = KEEP + (j - 32) * 128 + for_k1
            valid = e < 2 * KEEP
        Ap[j * 128 + for_k1[valid]] = Af[e[valid]]
    # SBUF layout ap_w[k1, j*512 + o]
    apw = Ap.reshape(64, 128, 512).transpose(1, 0, 2).reshape(128, 64 * 512)
    cvec = (ln_w[None, :] * W).sum(axis=1)                      # [512]
    dvec = ln_b @ W.T + b                                       # [512]
    cb = np.tile(cvec.astype(np.float32)[None, :], (128, 1))
    db = np.tile(dvec.astype(np.float32)[None, :], (128, 1))
    return apw.astype(BF16), cb, db


# ---------------------------------------------------------------- bass kernel
def build_nc(rows, block, reps=1):
    """Build the per-core Bass program for `rows` batch rows, processed in
    groups of `block` (the LN/linear batch tile, <= 128). reps>1 repeats the
    whole computation back-to-back (for HW timing)."""
    assert rows % block == 0
    nblk = rows // block
    nc = bacc.Bacc("TRN2", target_bir_lowering=False, debug=False)

    xd = nc.dram_tensor("x", [rows // 2, 128, 1024], f32r, kind="ExternalInput")
    f1d = nc.dram_tensor("f1", [2, 128, 256], f32r, kind="ExternalInput")
    tcsd = nc.dram_tensor("tcs", [128, 1536], bf16, kind="ExternalInput")
    wqd = nc.dram_tensor("wqk", [128, 1536], bf16, kind="ExternalInput")
    revd = nc.dram_tensor("rev2", [128, 256], bf16, kind="ExternalInput")
    apwd = nc.dram_tensor("apw", [128, 64 * 512], bf16, kind="ExternalInput")
    cd = nc.dram_tensor("cvec", [128, 512], f32, kind="ExternalInput")
    dd = nc.dram_tensor("dvec", [128, 512], f32, kind="ExternalInput")
    outd = nc.dram_tensor("out", [nblk, block, 512], f32, kind="ExternalOutput")

    from contextlib import ExitStack
    with tile.TileContext(nc) as tc, ExitStack() as es:
        consts = es.enter_context(tc.tile_pool(name="consts", bufs=1))
        f1a_sb = consts.tile([128, 256], f32r, name="f1a_sb")
        f1b_sb = consts.tile([128, 256], f32r, name="f1b_sb")
        tcs_sb = consts.tile([128, 1536], bf16, name="tcs_sb")
        wq_sb = consts.tile([128, 1536], bf16, name="wq_sb")
        rev_sb = consts.tile([128, 256], bf16, name="rev_sb")
        apw_sb = consts.tile([128, 64 * 512], bf16, name="apw_sb")
        c_sb = consts.tile([128, 512], f32, name="c_sb")
        d_sb = consts.tile([128, 512], f32, name="d_sb")
        ones_sb = consts.tile([128, 1], f32, name="ones_sb")
        ones_bf = consts.tile([128, 1], bf16, name="ones_bf")
        # keep the SP queue exclusively for x row loads: f1 (needed by the
        # first matmul, tiny) leads, medium consts ride the Activation HWDGE
        # queue, and the big linear weights (needed ~100us later) go via
        # gpsimd SWDGE
        nc.sync.dma_start(out=f1a_sb, in_=f1d[0])
        nc.sync.dma_start(out=f1b_sb, in_=f1d[1])
        for sb, dr in ((tcs_sb, tcsd), (wq_sb, wqd), (rev_sb, revd)):
            nc.scalar.dma_start(out=sb, in_=dr[:])
        for sb, dr in ((apw_sb, apwd), (c_sb, cd), (d_sb, dd)):
            nc.gpsimd.dma_start(out=sb, in_=dr[:])
        nc.vector.memset(ones_sb, 1.0)
        nc.vector.memset(ones_bf, 1.0)

        xp = es.enter_context(tc.tile_pool(name="xp", bufs=6))
        yp = es.enter_context(tc.tile_pool(name="yp", bufs=2, space="PSUM"))
        ybp = es.enter_context(tc.tile_pool(name="ybp", bufs=3))
        mp = es.enter_context(tc.tile_pool(name="mp", bufs=2))
        op1 = es.enter_context(tc.tile_pool(name="op1", bufs=1, space="PSUM"))
        pm = es.enter_context(tc.tile_pool(name="pm", bufs=1, space="PSUM"))
        svp = es.enter_context(tc.tile_pool(name="svp", bufs=2))
        sp = es.enter_context(tc.tile_pool(name="sp", bufs=2))
        sqp = es.enter_context(tc.tile_pool(name="sqp", bufs=1))
        stp = es.enter_context(tc.tile_pool(name="stp", bufs=2))
        gp = es.enter_context(tc.tile_pool(name="gp", bufs=1, space="PSUM"))
        g2p = es.enter_context(tc.tile_pool(name="g2p", bufs=1, space="PSUM"))
        smp = es.enter_context(tc.tile_pool(name="smp", bufs=2))
        ep = es.enter_context(tc.tile_pool(name="ep", bufs=2))

        apw3 = apw_sb.rearrange("p (j o) -> p j o", j=64)
        tc3 = tcs_sb.rearrange("p (v c k) -> p v c k", v=3, c=4)

        pbk = block // 2  # pairs per block

        def _tail_pieces(blk, s_buf, g_ps, g2):
            """Stats + LN + SiLU + output DMA for a finished block, split into
            small pieces so they interleave with the next block's pairs.
            sum(s^2) = DVE squares (bf16 2x) + an N=1 matmul series on PE
            accumulating over k1 and j at once (pm bank is free then)."""
            sq_hold = {}
            stat_hold = {}

            def sq_half(half):
                sq = sqp.tile([128, 32 * block], bf16, name="sq_buf")
                sq_hold[half] = sq
                sh = s_buf[:, half * 32 * block:(half + 1) * 32 * block]
                nc.vector.tensor_mul(sq, sh, sh)

            def sq_stats(half):
                if half == 0:
                    stat_hold[0] = pm.tile([128, 4], f32, name="stat_ps",
                                           tag="psm")
                stat_ps = stat_hold[0]
                sq = sq_hold[half]
                for jj in range(32):
                    nc.tensor.matmul(
                        stat_ps[:block, 0:1],
                        lhsT=sq[:, jj * block:(jj + 1) * block], rhs=ones_bf,
                        start=(half == 0 and jj == 0),
                        stop=(half == 1 and jj == 31))

            def finale():
                stat_ps = stat_hold[0]
                mu = smp.tile([128, 1], f32, name="mu")
                negmu = smp.tile([128, 1], f32, name="negmu")
                e2 = smp.tile([128, 1], f32, name="e2")
                varep = smp.tile([128, 1], f32, name="varep")
                rec = smp.tile([128, 1], f32, name="rec")
                istd = smp.tile([128, 1], f32, name="istd")
                nc.vector.tensor_scalar_mul(mu[:block], g2[:block],
                                            1.0 / (2 * KEEP))
                nc.vector.tensor_scalar_mul(negmu[:block], g2[:block],
                                            -1.0 / (2 * KEEP))
                nc.vector.tensor_scalar_mul(e2[:block], stat_ps[:block, 0:1],
                                            1.0 / (2 * KEEP))
                # varep = e2 - mu^2 + EPS = (mu * -mu) + e2, then + EPS
                nc.vector.scalar_tensor_tensor(
                    out=varep[:block], in0=mu[:block], scalar=negmu[:block],
                    in1=e2[:block], op0=ALU.mult, op1=ALU.add)
                nc.vector.tensor_scalar_add(varep[:block], varep[:block], EPS)
                nc.vector.reciprocal(rec[:block], varep[:block])
                nc.scalar.activation(istd[:block], rec[:block], ACT.Sqrt)
                p1 = ep.tile([128, 512], f32, name="p1")
                p2 = ep.tile([128, 512], f32, name="p2")
                o_sb = ep.tile([128, 512], f32, name="o_sb")
                nc.vector.scalar_tensor_tensor(
                    out=p1[:block], in0=c_sb[:block], scalar=negmu[:block],
                    in1=g_ps[:block], op0=ALU.mult, op1=ALU.add)
                nc.vector.scalar_tensor_tensor(
                    out=p2[:block], in0=p1[:block], scalar=istd[:block],
                    in1=d_sb[:block], op0=ALU.mult, op1=ALU.add)
                nc.scalar.activation(o_sb[:block], p2[:block], ACT.Silu)
                nc.scalar.dma_start(out=outd[blk], in_=o_sb[:block])

            return [lambda: sq_half(0), lambda: sq_stats(0),
                    lambda: sq_half(1), lambda: sq_stats(1), finale]

        import contextlib
        loop_ctx = tc.For_i(0, reps, 1) if reps > 1 else contextlib.nullcontext()
        with loop_ctx:
          pending = iter(())
          for blk in range(nblk):
              sv_buf = svp.tile([128, 128 * pbk], bf16, name="sv_buf")
              sv4 = sv_buf.rearrange("p (jq t) -> p jq t", jq=128)
              s_buf = sp.tile([128, 64 * block], bf16, name="s_buf")
              s4 = s_buf.rearrange("p (j t u) -> p j t u", j=64, u=2)
              for p in range(pbk):
                  if p >= 6 and (p - 6) % 8 == 0:
                      piece = next(pending, None)
                      if piece is not None:
                          piece()
                  xab = xp.tile([128, 1024], f32r, name="xab")
                  nc.sync.dma_start(out=xab, in_=xd[blk * pbk + p])
                  y_ps = yp.tile([128, 1024], f32, name="y_ps")
                  for c in range(4):
                      reg = y_ps[:, c * 256:(c + 1) * 256]
                      nc.tensor.matmul(reg, lhsT=xab[:, c * 128:(c + 1) * 128],
                                       rhs=f1a_sb, start=True, stop=False)
                      nc.tensor.matmul(reg,
                                       lhsT=xab[:, 512 + c * 128:512 + (c + 1) * 128],
                                       rhs=f1b_sb, start=False, stop=True)
                  y_bf = ybp.tile([128, 1024], bf16, name="y_bf")
                  nc.scalar.copy(out=y_bf, in_=y_ps)
                  y4 = y_bf.rearrange("p (c t k) -> p c t k", c=4, t=2)
                  yre = y4[:, :, 0, :]
                  yim = y4[:, :, 1, :]
                  m1 = mp.tile([128, 512], bf16, name="m1")
                  m2 = mp.tile([128, 512], bf16, name="m2")
                  m3 = mp.tile([128, 512], bf16, name="m3")
                  msum = mp.tile([128, 512], bf16, name="msum")
                  m1_3 = m1.rearrange("p (c k) -> p c k", c=4)
                  m2_3 = m2.rearrange("p (c k) -> p c k", c=4)
                  m3_3 = m3.rearrange("p (c k) -> p c k", c=4)
                  ms_3 = msum.rearrange("p (c k) -> p c k", c=4)
                  nc.vector.tensor_mul(m1_3, yre, tc3[:, 0])
                  nc.vector.tensor_mul(m2_3, yim, tc3[:, 1])
                  nc.vector.tensor_add(ms_3, yre, yim)
                  nc.gpsimd.tensor_mul(m3_3, ms_3, tc3[:, 2])
                  # outer DFT (64 q: 0..31 and 480..511), 12-matmul accumulation
                  # o cols = [re-lo 0:32 | re-hi 32:64 | im-lo 64:96 | im-hi 96:128]
                  osl = op1.tile([128, 128], f32, name="o_t")
                  # m1/m2 (DVE-computed) first; slower Pool m3 accumulates last
                  for mi, mt in ((0, m1), (1, m2)):
                      for c in range(4):
                          nc.tensor.matmul(
                              osl, lhsT=mt[:, c * 128:(c + 1) * 128],
                              rhs=wq_sb[:, c * 384 + mi * 128:c * 384 + mi * 128 + 128],
                              start=(mi == 0 and c == 0), stop=False)
                  for c in range(4):
                      nc.tensor.matmul(
                          osl, lhsT=m3[:, c * 128:(c + 1) * 128],
                          rhs=wq_sb[:, c * 384 + 256:c * 384 + 384],
                          start=False, stop=(c == 3))
                  nc.scalar.copy(out=sv4[:, :, p:p + 1], in_=osl.unsqueeze(2))
              # ---- separation: s_a = (X[k] + conj(X[-k]))/2, s_b = (X[k] -
              # conj(X[-k]))/(2i). Two shared mirror terms per q:
              #   M_re = 0.5*R@sv[63-q] + 0.5*S0@sv[64-q or 0]
              #   M_im = 0.5*R@sv[127-q] + 0.5*S0@sv[128-q or 64]
              # a_re = 0.5 sv[q] + M_re       a_im = 0.5 sv[64+q] - M_im
              # b_re = 0.5 sv[64+q] + M_im    b_im = -0.5 sv[q] + M_re
              RP, S0P = rev_sb[:, 0:128], rev_sb[:, 128:256]
              # linear psum (G) + sum(s) riding the same stationary as an N=1
              # matmul; the j-chunks are emitted interleaved with the sep
              # groups that produce them so PE fills the sep STT latency
              g_ps = gp.tile([128, 512], f32, name="g_ps")
              g2 = g2p.tile([128, 1], f32, name="g2")
              for g in range(9):
                if g < 8:
                  q0 = 4 * g
                  psm = pm.tile([128, 8 * pbk], f32, name="psm", tag="psm")
                  # M_re(q) and M_im(q) share lhsT and their sv columns sit 64
                  # jq apart -> one stride-64 rhs AP covers both (N=2*pbk);
                  # one pending accumulation group per psum bank at a time:
                  # R (start) then its S0 (stop) before the next region
                  for t in range(4):
                      q = q0 + t
                      reg = psm[:, t * 2 * pbk:(t + 1) * 2 * pbk]
                      nc.tensor.matmul(reg, lhsT=RP,
                                       rhs=sv4[:, 63 - q::64, :],
                                       start=True, stop=False)
                      nc.tensor.matmul(reg, lhsT=S0P,
                                       rhs=sv4[:, (0 if q == 0 else 64 - q)::64, :],
                                       start=False, stop=True)
                  psm5 = psm.rearrange("p (t m b) -> p t m b", t=4, m=2)
                  psm_re = psm5[:, :, 0, :].unsqueeze(3)
                  psm_im = psm5[:, :, 1, :].unsqueeze(3)
                  nc.vector.scalar_tensor_tensor(
                      out=s4[:, q0:q0 + 4, :, 0:1],
                      in0=sv4[:, q0:q0 + 4, :].unsqueeze(3), scalar=0.5,
                      in1=psm_re, op0=ALU.mult, op1=ALU.add)
                  nc.vector.scalar_tensor_tensor(
                      out=s4[:, 32 + q0:32 + q0 + 4, :, 0:1],
                      in0=sv4[:, 64 + q0:64 + q0 + 4, :].unsqueeze(3), scalar=0.5,
                      in1=psm_im, op0=ALU.mult, op1=ALU.subtract)
                  nc.vector.scalar_tensor_tensor(
                      out=s4[:, q0:q0 + 4, :, 1:2],
                      in0=sv4[:, 64 + q0:64 + q0 + 4, :].unsqueeze(3), scalar=0.5,
                      in1=psm_im, op0=ALU.mult, op1=ALU.add)
                  nc.vector.scalar_tensor_tensor(
                      out=s4[:, 32 + q0:32 + q0 + 4, :, 1:2],
                      in0=sv4[:, q0:q0 + 4, :].unsqueeze(3), scalar=-0.5,
                      in1=psm_re, op0=ALU.mult, op1=ALU.add)
                  if g == 7:
                      # mask bins k >= 4000 (q = 31, k1 >= 32), re and im;
                      # gpsimd keeps these off the busier DVE queue
                      for pbase in (32, 64, 96):
                          nc.gpsimd.memset(
                              s_buf[pbase:pbase + 32, 31 * block:32 * block], 0.0)
                          nc.gpsimd.memset(
                              s_buf[pbase:pbase + 32, 63 * block:64 * block], 0.0)
                if g >= 1:
                  # linear j-chunk the PREVIOUS sep group completed, staggered
                  # so PE never waits on this group's STTs (emission order
                  # makes j=0 the accumulation start and j=63 the stop)
                  q0p = 4 * (g - 1)
                  for j in (list(range(q0p, q0p + 4))
                            + list(range(32 + q0p, 32 + q0p + 4))):
                      sj = s_buf[:, j * block:(j + 1) * block]
                      nc.tensor.matmul(g_ps[:block], lhsT=sj, rhs=apw3[:, j, :],
                                       start=(j == 0), stop=(j == 63))
                      nc.tensor.matmul(g2[:block], lhsT=sj, rhs=ones_bf,
                                       start=(j == 0), stop=(j == 63))
              # defer this block's stats/LN tail: emitted in small pieces
              # between the NEXT block's pairs so the DVE queue never bursts
              pending = iter(_tail_pieces(blk, s_buf, g_ps, g2))
          for piece in pending:
              piece()

    nc.compile()
    return nc


# ---------------------------------------------------------------- entry points
_CACHE = {}


def _get_nc(rows, block, reps=1):
    key = (rows, block, reps)
    if key not in _CACHE:
        _CACHE[key] = build_nc(rows, block, reps)
    return _CACHE[key]


def run_cores(x, ln_w, ln_b, W, b, rows_per_core, block, n_cores=N_CORES,
              trace=False):
    f1, tcs, wqk, rev2 = _host_consts()
    apw, cb, db = _host_linear(
        np.asarray(ln_w, np.float64), np.asarray(ln_b, np.float64),
        np.asarray(W, np.float64), np.asarray(b, np.float64))
    nc = _get_nc(rows_per_core, block)
    x = np.ascontiguousarray(np.asarray(x, np.float32))
    in_maps = []
    for i in range(n_cores):
        xs = x[i * rows_per_core:(i + 1) * rows_per_core].reshape(
            rows_per_core // 2, 2, 128, 512).transpose(0, 2, 1, 3).reshape(
            rows_per_core // 2, 128, 1024)
        in_maps.append({
            "x": np.ascontiguousarray(xs), "f1": f1, "tcs": tcs, "wqk": wqk,
            "rev2": rev2, "apw": apw, "cvec": cb, "dvec": db,
        })
    res = run_bass_kernel_spmd(nc, in_maps, core_ids=list(range(n_cores)),
                               trace=trace)
    outs = [res.results[i]["out"].reshape(rows_per_core, 512)
            for i in range(n_cores)]
    return np.concatenate(outs, axis=0), res


def kernel(x, ln_w, ln_b, W, b):
    rows = B_FULL // N_CORES
    out, _ = run_cores(x, ln_w, ln_b, W, b, rows, 128)
    return out.reshape(B_FULL, 1, 512).astype(np.float32)


# revision 28
# speedup vs baseline: 1.4347x; 1.0200x over previous
"""Trainium2 Bass kernel: rFFT(65536)->keep 4000 bins->LayerNorm(8000)->Linear(8000,512)->SiLU.

Math: 2-level pruned Cooley-Tukey. n = 512*n1 + n2 (n1 in [0,128), n2 in [0,512)).
  k1 = k mod 128, q = k div 128; keep k < 4000 -> q in [0,32), mask (q=31, k1>=32).
  Pair rows (xa, xb) as complex z = xa + i*xb:
  Y[n2,k1]  = sum_n1 z[512*n1+n2] * exp(-2i pi n1 k1 / 128)        (inner DFT, f32r matmul)
  Twiddle T[n2,k1] = exp(-2i pi n2 k1 / 65536) and outer DFT over n2 fused via
  3-mult Karatsuba: m1 = Yre*Tc, m2 = Yim*Ts, m3 = (Yre+Yim)*(Tc+Ts) (DVE/Pool), then
  X[q,k1] = sum_n2 m1*(Wc+Ws | Ws-Wc) + m2*(Ws-Wc | -(Ws+Wc)) + m3*(-Ws | Wc)
  for 64 q (0..31 and 480..511), 12 matmul accumulation steps (was 16).
  Separation uses 2 shared mirror terms per q (a_re/b_im share M_re, a_im/b_re
  share M_im); M_re/M_im ride one stride-64 rhs AP -> 2 matmuls per q, then 4
  fused STTs per 4-q group. The linear j-chunks are emitted staggered into the
  separation groups that produce them, and each block's stats/LN/SiLU tail is
  deferred and drip-fed between the next block's pairs so no engine bursts.
  LayerNorm folded into the linear:
  out = SiLU( (G - mu*c) * istd + d ),  G = s @ A',  A'[e,o] = ln_w[e]*W[o,e],
  c = sum_e ln_w*W, d = ln_b @ W.T + b. sum(s) and sum(s^2) ride N=1 matmul
  series (sum(s) on the linear's stationary; sum(s^2) on DVE-squared tiles).
"""

import numpy as np
import ml_dtypes

import concourse.bass as bass
import concourse.tile as tile
from concourse import bacc, mybir
from concourse.bass_utils import run_bass_kernel_spmd

N_CORES = 8
B_FULL = 2048
FFT_N = 65536
N1 = 128      # inner DFT length; k1 = k mod 128
N2 = 512      # outer length; q = k div 128
KEEP = 4000
QK = 32       # q in [0, 32), 32*128 = 4096 bins computed, 96 masked
EPS = 1e-5

f32 = mybir.dt.float32
f32r = mybir.dt.float32r
bf16 = mybir.dt.bfloat16
ALU = mybir.AluOpType
ACT = mybir.ActivationFunctionType
BF16 = ml_dtypes.bfloat16


# ---------------------------------------------------------------- host consts
def _host_consts():
    n1 = np.arange(N1, dtype=np.float64)
    k1 = np.arange(N1, dtype=np.float64)
    n2 = np.arange(N2, dtype=np.float64)

    ang1 = 2.0 * np.pi * np.outer(n1, k1) / N1
    C1, S1 = np.cos(ang1), -np.sin(ang1)
    # pair-packed inner: y = [Yre | Yim] accumulates
    #   lhsT=xa with [C1 | S1]  plus  lhsT=xb with [-S1 | C1]
    f1 = np.concatenate([np.concatenate([C1, S1], axis=1),
                         np.concatenate([-S1, C1], axis=1)], axis=0).reshape(
        2, 128, 256)  # f1[0]=for xa, f1[1]=for xb

    angT = 2.0 * np.pi * np.outer(n2, k1) / FFT_N               # [512,128]
    Tc, Ts = np.cos(angT), -np.sin(angT)
    # chunk-major twiddle tables [n2', c*128 + k1]: Tc | Ts | Tc+Ts
    tcs = np.zeros((128, 3 * 512))
    for c in range(4):
        sl = slice(c * 128, (c + 1) * 128)
        tcs[:, 0 + c * 128:0 + (c + 1) * 128] = Tc[sl]
        tcs[:, 512 + c * 128:512 + (c + 1) * 128] = Ts[sl]
        tcs[:, 1024 + c * 128:1024 + (c + 1) * 128] = Tc[sl] + Ts[sl]

    qall = np.concatenate([np.arange(QK), np.arange(N2 - QK, N2)])  # 64 q values
    angW = 2.0 * np.pi * np.outer(n2, qall.astype(np.float64)) / N2  # [512,64]
    WcT, WsT = np.cos(angW), -np.sin(angW)
    # Karatsuba outer weights, 3 blocks of 128 cols (64 re | 64 im) per chunk
    wqk = np.zeros((128, 4 * 384))
    for c in range(4):
        Wc = WcT[c * 128:(c + 1) * 128]; Ws = WsT[c * 128:(c + 1) * 128]
        base = c * 384
        wqk[:, base + 0:base + 64] = Wc + Ws
        wqk[:, base + 64:base + 128] = Ws - Wc
        wqk[:, base + 128:base + 192] = Ws - Wc
        wqk[:, base + 192:base + 256] = -(Ws + Wc)
        wqk[:, base + 256:base + 320] = -Ws
        wqk[:, base + 320:base + 384] = Wc

    # separation mirror constants, scaled 0.5, as matmul lhsT [src_k1, dst_k1]
    R = np.zeros((128, 128)); S0 = np.zeros((128, 128))
    for d in range(1, 128):
        R[128 - d, d] = 1.0
    S0[0, 0] = 1.0
    rev2 = np.concatenate([0.5 * R, 0.5 * S0], axis=1)  # [128, 256]

    return (f1.astype(np.float32), tcs.astype(BF16), wqk.astype(BF16),
            rev2.astype(BF16))


def _host_linear(ln_w, ln_b, W, b):
    # A'[e,o] = ln_w[e] * W[o,e]; permuted to my (j,k1) layout with masked tail.
    Af = (ln_w[None, :] * W).T.astype(np.float64)               # [8000, 512]
    Ap = np.zeros((8192, 512))
    for j in range(64):
        for_k1 = np.arange(128)
        if j < 32:
            e = j * 128 + for_k1
            valid = e < KEEP
        else:
            e = KEEP + (j - 32) * 128 + for_k1
            valid = e < 2 * KEEP
        Ap[j * 128 + for_k1[valid]] = Af[e[valid]]
    # SBUF layout ap_w[k1, j*512 + o]
    apw = Ap.reshape(64, 128, 512).transpose(1, 0, 2).reshape(128, 64 * 512)
    cvec = (ln_w[None, :] * W).sum(axis=1)                      # [512]
    dvec = ln_b @ W.T + b                                       # [512]
    cb = np.tile(cvec.astype(np.float32)[None, :], (128, 1))
    db = np.tile(dvec.astype(np.float32)[None, :], (128, 1))
    return apw.astype(BF16), cb, db


# ---------------------------------------------------------------- bass kernel
def build_nc(rows, block, reps=1):
    """Build the per-core Bass program for `rows` batch rows, processed in
    groups of `block` (the LN/linear batch tile, <= 128). reps>1 repeats the
    whole computation back-to-back (for HW timing)."""
    assert rows % block == 0
    nblk = rows // block
    nc = bacc.Bacc("TRN2", target_bir_lowering=False, debug=False)

    xd = nc.dram_tensor("x", [rows // 2, 128, 1024], f32r, kind="ExternalInput")
    f1d = nc.dram_tensor("f1", [2, 128, 256], f32r, kind="ExternalInput")
    tcsd = nc.dram_tensor("tcs", [128, 1536], bf16, kind="ExternalInput")
    wqd = nc.dram_tensor("wqk", [128, 1536], bf16, kind="ExternalInput")
    revd = nc.dram_tensor("rev2", [128, 256], bf16, kind="ExternalInput")
    apwd = nc.dram_tensor("apw", [128, 64 * 512], bf16, kind="ExternalInput")
    cd = nc.dram_tensor("cvec", [128, 512], f32, kind="ExternalInput")
    dd = nc.dram_tensor("dvec", [128, 512], f32, kind="ExternalInput")
    outd = nc.dram_tensor("out", [nblk, block, 512], f32, kind="ExternalOutput")

    from contextlib import ExitStack
    with tile.TileContext(nc) as tc, ExitStack() as es:
        consts = es.enter_context(tc.tile_pool(name="consts", bufs=1))
        f1a_sb = consts.tile([128, 256], f32r, name="f1a_sb")
        f1b_sb = consts.tile([128, 256], f32r, name="f1b_sb")
        tcs_sb = consts.tile([128, 1536], bf16, name="tcs_sb")
        wq_sb = consts.tile([128, 1536], bf16, name="wq_sb")
        rev_sb = consts.tile([128, 256], bf16, name="rev_sb")
        apw_sb = consts.tile([128, 64 * 512], bf16, name="apw_sb")
        c_sb = consts.tile([128, 512], f32, name="c_sb")
        d_sb = consts.tile([128, 512], f32, name="d_sb")
        ones_bf = consts.tile([128, 1], bf16, name="ones_bf")
        # keep the SP queue exclusively for x row loads: f1 (needed by the
        # first matmul, tiny) leads, medium consts ride the Activation HWDGE
        # queue, and the big linear weights (needed ~100us later) go via
        # gpsimd SWDGE
        nc.sync.dma_start(out=f1a_sb, in_=f1d[0])
        nc.sync.dma_start(out=f1b_sb, in_=f1d[1])
        for sb, dr in ((tcs_sb, tcsd), (wq_sb, wqd), (rev_sb, revd)):
            nc.scalar.dma_start(out=sb, in_=dr[:])
        for sb, dr in ((apw_sb, apwd), (c_sb, cd), (d_sb, dd)):
            nc.gpsimd.dma_start(out=sb, in_=dr[:])
        nc.vector.memset(ones_bf, 1.0)

        xp = es.enter_context(tc.tile_pool(name="xp", bufs=8))
        yp = es.enter_context(tc.tile_pool(name="yp", bufs=2, space="PSUM"))
        ybp = es.enter_context(tc.tile_pool(name="ybp", bufs=3))
        mp = es.enter_context(tc.tile_pool(name="mp", bufs=2))
        op1 = es.enter_context(tc.tile_pool(name="op1", bufs=1, space="PSUM"))
        pm = es.enter_context(tc.tile_pool(name="pm", bufs=1, space="PSUM"))
        svp = es.enter_context(tc.tile_pool(name="svp", bufs=2))
        sp = es.enter_context(tc.tile_pool(name="sp", bufs=2))
        sqp = es.enter_context(tc.tile_pool(name="sqp", bufs=1))
        stp = es.enter_context(tc.tile_pool(name="stp", bufs=2))
        gp = es.enter_context(tc.tile_pool(name="gp", bufs=1, space="PSUM"))
        g2p = es.enter_context(tc.tile_pool(name="g2p", bufs=1, space="PSUM"))
        smp = es.enter_context(tc.tile_pool(name="smp", bufs=2))
        ep = es.enter_context(tc.tile_pool(name="ep", bufs=2))

        apw3 = apw_sb.rearrange("p (j o) -> p j o", j=64)
        tc3 = tcs_sb.rearrange("p (v c k) -> p v c k", v=3, c=4)

        pbk = block // 2  # pairs per block

        def _tail_pieces(blk, s_buf, g_ps, g2):
            """Stats + LN + SiLU + output DMA for a finished block, split into
            small pieces so they interleave with the next block's pairs.
            sum(s^2) = DVE squares (bf16 2x) + an N=1 matmul series on PE
            accumulating over k1 and j at once (pm bank is free then)."""
            sq_hold = {}
            stat_hold = {}

            def sq_half(half):
                sq = sqp.tile([128, 32 * block], bf16, name="sq_buf")
                sq_hold[half] = sq
                sh = s_buf[:, half * 32 * block:(half + 1) * 32 * block]
                nc.vector.tensor_mul(sq, sh, sh)

            def sq_stats(half):
                if half == 0:
                    stat_hold[0] = pm.tile([128, 4], f32, name="stat_ps",
                                           tag="psm")
                stat_ps = stat_hold[0]
                sq = sq_hold[half]
                for jj in range(32):
                    nc.tensor.matmul(
                        stat_ps[:block, 0:1],
                        lhsT=sq[:, jj * block:(jj + 1) * block], rhs=ones_bf,
                        start=(half == 0 and jj == 0),
                        stop=(half == 1 and jj == 31))

            def finale():
                stat_ps = stat_hold[0]
                mu = smp.tile([128, 1], f32, name="mu")
                negmu = smp.tile([128, 1], f32, name="negmu")
                e2 = smp.tile([128, 1], f32, name="e2")
                varep = smp.tile([128, 1], f32, name="varep")
                rec = smp.tile([128, 1], f32, name="rec")
                istd = smp.tile([128, 1], f32, name="istd")
                nc.vector.tensor_scalar_mul(mu[:block], g2[:block],
                                            1.0 / (2 * KEEP))
                nc.vector.tensor_scalar_mul(negmu[:block], g2[:block],
                                            -1.0 / (2 * KEEP))
                nc.vector.tensor_scalar_mul(e2[:block], stat_ps[:block, 0:1],
                                            1.0 / (2 * KEEP))
                # varep = e2 - mu^2 + EPS = (mu * -mu) + e2, then + EPS
                nc.vector.scalar_tensor_tensor(
                    out=varep[:block], in0=mu[:block], scalar=negmu[:block],
                    in1=e2[:block], op0=ALU.mult, op1=ALU.add)
                nc.vector.tensor_scalar_add(varep[:block], varep[:block], EPS)
                nc.vector.reciprocal(rec[:block], varep[:block])
                nc.scalar.activation(istd[:block], rec[:block], ACT.Sqrt)
                p1 = ep.tile([128, 512], f32, name="p1")
                p2 = ep.tile([128, 512], f32, name="p2")
                o_sb = ep.tile([128, 512], f32, name="o_sb")
                nc.vector.scalar_tensor_tensor(
                    out=p1[:block], in0=c_sb[:block], scalar=negmu[:block],
                    in1=g_ps[:block], op0=ALU.mult, op1=ALU.add)
                nc.vector.scalar_tensor_tensor(
                    out=p2[:block], in0=p1[:block], scalar=istd[:block],
                    in1=d_sb[:block], op0=ALU.mult, op1=ALU.add)
                nc.scalar.activation(o_sb[:block], p2[:block], ACT.Silu)
                nc.scalar.dma_start(out=outd[blk], in_=o_sb[:block])

            return [lambda: sq_half(0), lambda: sq_stats(0),
                    lambda: sq_half(1), lambda: sq_stats(1), finale]

        import contextlib
        loop_ctx = tc.For_i(0, reps, 1) if reps > 1 else contextlib.nullcontext()
        with loop_ctx:
          pending = iter(())
          for blk in range(nblk):
              sv_buf = svp.tile([128, 128 * pbk], bf16, name="sv_buf")
              sv4 = sv_buf.rearrange("p (jq t) -> p jq t", jq=128)
              s_buf = sp.tile([128, 64 * block], bf16, name="s_buf")
              s4 = s_buf.rearrange("p (j t u) -> p j t u", j=64, u=2)
              for p in range(pbk):
                  if p >= 6 and (p - 6) % 8 == 0:
                      piece = next(pending, None)
                      if piece is not None:
                          piece()
                  xab = xp.tile([128, 1024], f32r, name="xab")
                  nc.sync.dma_start(out=xab, in_=xd[blk * pbk + p])
                  y_ps = yp.tile([128, 1024], f32, name="y_ps")
                  for c in range(4):
                      reg = y_ps[:, c * 256:(c + 1) * 256]
                      nc.tensor.matmul(reg, lhsT=xab[:, c * 128:(c + 1) * 128],
                                       rhs=f1a_sb, start=True, stop=False)
                      nc.tensor.matmul(reg,
                                       lhsT=xab[:, 512 + c * 128:512 + (c + 1) * 128],
                                       rhs=f1b_sb, start=False, stop=True)
                  y_bf = ybp.tile([128, 1024], bf16, name="y_bf")
                  nc.scalar.copy(out=y_bf, in_=y_ps)
                  y4 = y_bf.rearrange("p (c t k) -> p c t k", c=4, t=2)
                  yre = y4[:, :, 0, :]
                  yim = y4[:, :, 1, :]
                  m1 = mp.tile([128, 512], bf16, name="m1")
                  m2 = mp.tile([128, 512], bf16, name="m2")
                  m3 = mp.tile([128, 512], bf16, name="m3")
                  msum = mp.tile([128, 512], bf16, name="msum")
                  m1_3 = m1.rearrange("p (c k) -> p c k", c=4)
                  m2_3 = m2.rearrange("p (c k) -> p c k", c=4)
                  m3_3 = m3.rearrange("p (c k) -> p c k", c=4)
                  ms_3 = msum.rearrange("p (c k) -> p c k", c=4)
                  nc.vector.tensor_mul(m1_3, yre, tc3[:, 0])
                  nc.vector.tensor_mul(m2_3, yim, tc3[:, 1])
                  nc.vector.tensor_add(ms_3, yre, yim)
                  nc.gpsimd.tensor_mul(m3_3, ms_3, tc3[:, 2])
                  # outer DFT (64 q: 0..31 and 480..511), 12-matmul accumulation
                  # o cols = [re-lo 0:32 | re-hi 32:64 | im-lo 64:96 | im-hi 96:128]
                  osl = op1.tile([128, 128], f32, name="o_t")
                  # m1/m2 (DVE-computed) first; slower Pool m3 accumulates last
                  for mi, mt in ((0, m1), (1, m2)):
                      for c in range(4):
                          nc.tensor.matmul(
                              osl, lhsT=mt[:, c * 128:(c + 1) * 128],
                              rhs=wq_sb[:, c * 384 + mi * 128:c * 384 + mi * 128 + 128],
                              start=(mi == 0 and c == 0), stop=False)
                  for c in range(4):
                      nc.tensor.matmul(
                          osl, lhsT=m3[:, c * 128:(c + 1) * 128],
                          rhs=wq_sb[:, c * 384 + 256:c * 384 + 384],
                          start=False, stop=(c == 3))
                  nc.scalar.copy(out=sv4[:, :, p:p + 1], in_=osl.unsqueeze(2))
              # ---- separation: s_a = (X[k] + conj(X[-k]))/2, s_b = (X[k] -
              # conj(X[-k]))/(2i). Two shared mirror terms per q:
              #   M_re = 0.5*R@sv[63-q] + 0.5*S0@sv[64-q or 0]
              #   M_im = 0.5*R@sv[127-q] + 0.5*S0@sv[128-q or 64]
              # a_re = 0.5 sv[q] + M_re       a_im = 0.5 sv[64+q] - M_im
              # b_re = 0.5 sv[64+q] + M_im    b_im = -0.5 sv[q] + M_re
              RP, S0P = rev_sb[:, 0:128], rev_sb[:, 128:256]
              # linear psum (G) + sum(s) riding the same stationary as an N=1
              # matmul; the j-chunks are emitted interleaved with the sep
              # groups that produce them so PE fills the sep STT latency
              g_ps = gp.tile([128, 512], f32, name="g_ps")
              g2 = g2p.tile([128, 1], f32, name="g2")
              for g in range(9):
                if g < 8:
                  q0 = 4 * g
                  psm = pm.tile([128, 8 * pbk], f32, name="psm", tag="psm")
                  # M_re(q) and M_im(q) share lhsT and their sv columns sit 64
                  # jq apart -> one stride-64 rhs AP covers both (N=2*pbk);
                  # one pending accumulation group per psum bank at a time:
                  # R (start) then its S0 (stop) before the next region
                  for t in range(4):
                      q = q0 + t
                      reg = psm[:, t * 2 * pbk:(t + 1) * 2 * pbk]
                      nc.tensor.matmul(reg, lhsT=RP,
                                       rhs=sv4[:, 63 - q::64, :],
                                       start=True, stop=False)
                      nc.tensor.matmul(reg, lhsT=S0P,
                                       rhs=sv4[:, (0 if q == 0 else 64 - q)::64, :],
                                       start=False, stop=True)
                  psm5 = psm.rearrange("p (t m b) -> p t m b", t=4, m=2)
                  psm_re = psm5[:, :, 0, :].unsqueeze(3)
                  psm_im = psm5[:, :, 1, :].unsqueeze(3)
                  nc.vector.scalar_tensor_tensor(
                      out=s4[:, q0:q0 + 4, :, 0:1],
                      in0=sv4[:, q0:q0 + 4, :].unsqueeze(3), scalar=0.5,
                      in1=psm_re, op0=ALU.mult, op1=ALU.add)
                  nc.vector.scalar_tensor_tensor(
                      out=s4[:, 32 + q0:32 + q0 + 4, :, 0:1],
                      in0=sv4[:, 64 + q0:64 + q0 + 4, :].unsqueeze(3), scalar=0.5,
                      in1=psm_im, op0=ALU.mult, op1=ALU.subtract)
                  nc.vector.scalar_tensor_tensor(
                      out=s4[:, q0:q0 + 4, :, 1:2],
                      in0=sv4[:, 64 + q0:64 + q0 + 4, :].unsqueeze(3), scalar=0.5,
                      in1=psm_im, op0=ALU.mult, op1=ALU.add)
                  nc.vector.scalar_tensor_tensor(
                      out=s4[:, 32 + q0:32 + q0 + 4, :, 1:2],
                      in0=sv4[:, q0:q0 + 4, :].unsqueeze(3), scalar=-0.5,
                      in1=psm_re, op0=ALU.mult, op1=ALU.add)
                  if g == 7:
                      # mask bins k >= 4000 (q = 31, k1 >= 32), re and im;
                      # gpsimd keeps these off the busier DVE queue
                      for pbase in (32, 64, 96):
                          nc.gpsimd.memset(
                              s_buf[pbase:pbase + 32, 31 * block:32 * block], 0.0)
                          nc.gpsimd.memset(
                              s_buf[pbase:pbase + 32, 63 * block:64 * block], 0.0)
                if g >= 1:
                  # linear j-chunk the PREVIOUS sep group completed, staggered
                  # so PE never waits on this group's STTs (emission order
                  # makes j=0 the accumulation start and j=63 the stop)
                  q0p = 4 * (g - 1)
                  for j in (list(range(q0p, q0p + 4))
                            + list(range(32 + q0p, 32 + q0p + 4))):
                      sj = s_buf[:, j * block:(j + 1) * block]
                      nc.tensor.matmul(g_ps[:block], lhsT=sj, rhs=apw3[:, j, :],
                                       start=(j == 0), stop=(j == 63))
                      nc.tensor.matmul(g2[:block], lhsT=sj, rhs=ones_bf,
                                       start=(j == 0), stop=(j == 63))
              # defer this block's stats/LN tail: emitted in small pieces
              # between the NEXT block's pairs so the DVE queue never bursts
              pending = iter(_tail_pieces(blk, s_buf, g_ps, g2))
          for piece in pending:
              piece()

    nc.compile()
    return nc


# ---------------------------------------------------------------- entry points
_CACHE = {}


def _get_nc(rows, block, reps=1):
    key = (rows, block, reps)
    if key not in _CACHE:
        _CACHE[key] = build_nc(rows, block, reps)
    return _CACHE[key]


def run_cores(x, ln_w, ln_b, W, b, rows_per_core, block, n_cores=N_CORES,
              trace=False):
    f1, tcs, wqk, rev2 = _host_consts()
    apw, cb, db = _host_linear(
        np.asarray(ln_w, np.float64), np.asarray(ln_b, np.float64),
        np.asarray(W, np.float64), np.asarray(b, np.float64))
    nc = _get_nc(rows_per_core, block)
    x = np.ascontiguousarray(np.asarray(x, np.float32))
    in_maps = []
    for i in range(n_cores):
        xs = x[i * rows_per_core:(i + 1) * rows_per_core].reshape(
            rows_per_core // 2, 2, 128, 512).transpose(0, 2, 1, 3).reshape(
            rows_per_core // 2, 128, 1024)
        in_maps.append({
            "x": np.ascontiguousarray(xs), "f1": f1, "tcs": tcs, "wqk": wqk,
            "rev2": rev2, "apw": apw, "cvec": cb, "dvec": db,
        })
    res = run_bass_kernel_spmd(nc, in_maps, core_ids=list(range(n_cores)),
                               trace=trace)
    outs = [res.results[i]["out"].reshape(rows_per_core, 512)
            for i in range(n_cores)]
    return np.concatenate(outs, axis=0), res


def kernel(x, ln_w, ln_b, W, b):
    rows = B_FULL // N_CORES
    out, _ = run_cores(x, ln_w, ln_b, W, b, rows, 128)
    return out.reshape(B_FULL, 1, 512).astype(np.float32)
